# revision 6
# baseline (speedup 1.0000x reference)
"""Trainium2 Bass kernel: fused embedding gather-concat.

out[r] = concat(proc_pos[process_ids[r]], locs_sp[location_ids[r]])   r in [0, 8M)

Sharding: rows data-parallel across 8 NeuronCores (1M rows each, padded to a
tile multiple); lookup tables replicated in every core's DRAM.

v2 (vs v1's 9.68ms): trace showed GPSIMD desc-gen is the wall: each
dma_gather generates 8192 SWDGE descriptors at ~7.4ns/desc on ONE Q7 cpu
pair (cpu_id/2 == queue_num), ~60.7us/gather, and the ring only drains at
gen end (+17us tail).  v1 ran loc+proc gathers per tile (two 60.7us gens)
and adjacent tiles shared a queue, serializing gen->drain->gen: 78.7us/tile.

  - proc part moved OFF gpsimd entirely: one-hot matmuls on the idle PE.
    Host ships, per tile, 8 stationary matrices OHW[G] in [128,128] f32:
    OHW[G][16g+k, p] = (pid[row(p, slot 8G+g)] == k).  rhs is a block-diag
    [128, 64] with proc_pos in 8 diagonal [16,8] blocks, so PSUM picks up
    ps[p, 8s+d] = proc_pos[pid(p,s), d] -- [128, S, 8] contiguous.  Exact
    in fp32 (single 1.0*value product per output).  ACT copies PSUM->outb.
  - loc gather (the only SWDGE user) rotates queues b%4, so up to 4 gens
    run concurrently on the 4 Q7 cpu pairs (Pool exec queue depth = 4) and
    a queue's ring drain overlaps other queues' gens.
  - DEPTH=6 slot ring so enough tiles are in flight to feed 4 queues.

Per-tile pipeline: SP loads eidx/off/ohw + stores out tiles; gpsimd issues
the loc gather (queue b%4); PE runs 8 one-hot matmuls into the tile's PSUM
bank; ACT copies PSUM into outb[:, :, 0:8]; DVE builds the 16 offset masks
and runs 16 copy_predicated extracting 12B/row from the gathered 256B
blocks into outb[:, :, 8:11].
"""

from contextlib import ExitStack

import numpy as np

import concourse.bass as bass
import concourse.mybir as mybir
from concourse import library_config
from concourse.bass_utils import run_bass_kernel_spmd

N_CORES = 8
NAUG = 8_000_000
PER_CORE = NAUG // N_CORES  # 1,000,000

NUM_PROCS = 16
PROC_DIM = 8
NUM_LOCS = 500_000
SPATIAL_DIM = 3
OUT_DIM = PROC_DIM + SPATIAL_DIM  # 11

NBLK = NUM_LOCS // 16  # 31250 blocks of 16 padded rows each

T = 8192  # rows per tile (= indices per dma_gather)
S = T // 128  # slots per partition (64)
NT = -(-PER_CORE // T)  # 123
N_PAD = NT * T  # 1,007,616
DEPTH = 7
ODEPTH = 14  # outb/store ring is deeper so ACT's proc writes commit long before DVE's loc writes hit the same rows
NQ = 4  # SWDGE queues


def build_nc():
    nc = bass.Bass(num_swdge_queues=NQ)
    loc64 = nc.declare_dram_parameter("loc64", [NBLK, 64], mybir.dt.float32, isOutput=False)
    pmat_d = nc.declare_dram_parameter("pmat", [128, 64], mybir.dt.float32, isOutput=False)
    iota16 = nc.declare_dram_parameter("iota16", [128, 16], mybir.dt.int32, isOutput=False)
    eidx = nc.declare_dram_parameter("eidx", [NT, 128, T // 16], mybir.dt.int16, isOutput=False)
    ohw = nc.declare_dram_parameter("ohw", [NT, 128, 8 * 128], mybir.dt.float32, isOutput=False)
    off = nc.declare_dram_parameter("off", [NT, 128, S], mybir.dt.int32, isOutput=False)
    out = nc.declare_dram_parameter("out", [N_PAD, OUT_DIM], mybir.dt.float32, isOutput=True)
    out_v = out.rearrange("(b p s) d -> b p (s d)", b=NT, p=128)

    W = T // 16  # wrapped idx columns

    def uses(s):
        return (NT - s + DEPTH - 1) // DEPTH

    def uses14(od):
        return (NT - od + ODEPTH - 1) // ODEPTH

    with ExitStack() as ctx:
        ldi = ctx.enter_context(nc.semaphore("ldi"))
        ld = [ctx.enter_context(nc.semaphore(f"ld{s}")) for s in range(DEPTH)]
        g = [ctx.enter_context(nc.semaphore(f"g{s}")) for s in range(DEPTH)]
        m = [ctx.enter_context(nc.semaphore(f"m{s}")) for s in range(DEPTH)]
        v = [ctx.enter_context(nc.semaphore(f"v{s}")) for s in range(DEPTH)]
        st = [ctx.enter_context(nc.semaphore(f"st{s}")) for s in range(ODEPTH)]
        eidx_sb = ctx.enter_context(nc.sbuf_tensor("eidx_sb", [128, DEPTH * W], mybir.dt.int16))
        ohw_sb = ctx.enter_context(nc.sbuf_tensor("ohw_sb", [128, DEPTH * 8 * 128], mybir.dt.float32))
        off_sb = ctx.enter_context(nc.sbuf_tensor("off_sb", [128, DEPTH * S], mybir.dt.int32))
        iota_sb = ctx.enter_context(nc.sbuf_tensor("iota_sb", [128, 16], mybir.dt.int32))
        pmat_sb = ctx.enter_context(nc.sbuf_tensor("pmat_sb", [128, 64], mybir.dt.float32))
        gloc = ctx.enter_context(nc.sbuf_tensor("gloc", [128, DEPTH * S * 64], mybir.dt.float32))
        outb = ctx.enter_context(nc.sbuf_tensor("outb", [128, ODEPTH * S * OUT_DIM], mybir.dt.float32))
        mbuf = ctx.enter_context(nc.sbuf_tensor("mbuf", [128, DEPTH * S * 16], mybir.dt.uint8))
        ps = ctx.enter_context(nc.psum_tensor("ps", [128, DEPTH * S * 8], mybir.dt.float32))
        block = ctx.enter_context(nc.Block())

        def eslot(s):
            return eidx_sb[:, s * W:(s + 1) * W]

        def wslot(s, G):
            return ohw_sb[:, (s * 8 + G) * 128:(s * 8 + G + 1) * 128]

        def oslot(s):
            return off_sb[:, s * S:(s + 1) * S]

        def glslot(s):
            return gloc[:, s * S * 64:(s + 1) * S * 64].rearrange("p (s e) -> p s e", e=64)

        def psslot(s):
            return ps[:, s * S * 8:(s + 1) * S * 8].rearrange("p (s e) -> p s e", e=8)

        def oslot_out(od):
            return outb[:, od * S * OUT_DIM:(od + 1) * S * OUT_DIM].rearrange(
                "p (s e) -> p s e", e=OUT_DIM)

        def mslot(s):
            return mbuf[:, s * S * 16:(s + 1) * S * 16].rearrange("p (s e) -> p s e", e=16)

        def issue_loads(sp, b):
            s = b % DEPTH
            sp.dma_start(out=eslot(s), in_=eidx[b]).then_inc(ld[s], 16)
            sp.dma_start(out=off_sb[:, s * S:(s + 1) * S], in_=off[b]).then_inc(ld[s], 16)
            sp.dma_start(
                out=ohw_sb[:, s * 8 * 128:(s + 1) * 8 * 128], in_=ohw[b]
            ).then_inc(ld[s], 16)

        @block.sync
        def _(sp):
            sp.dma_start(out=iota_sb[:], in_=iota16[:]).then_inc(ldi, 16)
            sp.dma_start(out=pmat_sb[:], in_=pmat_d[:]).then_inc(ldi, 16)
            for b in range(min(DEPTH, NT)):
                issue_loads(sp, b)
            for b in range(NT):
                s, u = b % DEPTH, b // DEPTH
                od = b % ODEPTH
                sp.wait_ge(v[s], 2 * (u + 1))
                sp.dma_start(
                    out=out_v[b],
                    in_=outb[:, od * S * OUT_DIM:(od + 1) * S * OUT_DIM],
                ).then_inc(st[od], 16)
                if b + DEPTH < NT:
                    # slot reuse: eidx free once gather ran; ohw free once PE ran
                    sp.wait_ge(g[s], 16 * (u + 1))
                    sp.wait_ge(m[s], 8 * (u + 1))
                    issue_loads(sp, b + DEPTH)
            for od in range(ODEPTH):
                sp.wait_ge(st[od], 16 * uses14(od))

        @block.gpsimd
        def _(gp):
            gp.load_library(library_config.mlp)
            t_reg = gp.to_reg(T)
            for b in range(NT):
                s, u = b % DEPTH, b // DEPTH
                gp.wait_ge(ld[s], 48 * (u + 1))
                if u >= 1:
                    # gather dst slot free once the slot's extract+copy ran
                    gp.wait_ge(v[s], 2 * u)
                gp.dma_gather(
                    glslot(s), loc64[:], eslot(s), T, t_reg, 64,
                    queue_num=b % NQ, single_packet=False,
                ).then_inc(g[s], 16)
            for s in range(DEPTH):
                gp.wait_ge(g[s], 16 * uses(s))

        @block.tensor
        def _(pe):
            pe.wait_ge(ldi, 32)
            for b in range(NT):
                s, u = b % DEPTH, b // DEPTH
                pe.wait_ge(ld[s], 48 * (u + 1))
                if u >= 1:
                    # psum slot free once ACT copied the previous use
                    pe.wait_ge(v[s], 2 * u)
                for G in range(8):
                    pe.matmul(
                        ps[:, s * S * 8 + G * 64:s * S * 8 + (G + 1) * 64],
                        wslot(s, G),
                        pmat_sb[:],
                    ).then_inc(m[s], 1)

        @block.scalar
        def _(act):
            for b in range(NT):
                s, u = b % DEPTH, b // DEPTH
                od, u14 = b % ODEPTH, b // ODEPTH
                act.wait_ge(m[s], 8 * (u + 1))
                if u14 >= 1:
                    act.wait_ge(st[od], 16 * u14)
                act.copy(out=oslot_out(od)[:, :, 0:PROC_DIM], in_=psslot(s)).then_inc(v[s], 1)

        @block.vector
        def _(dve):
            dve.wait_ge(ldi, 32)

            def build_mask(b):
                # Mask build only needs off + iota (loaded DEPTH tiles ahead).
                s, u = b % DEPTH, b // DEPTH
                od, u14 = b % ODEPTH, b // ODEPTH
                dve.wait_ge(ld[s], 48 * (u + 1))
                if u14 >= 1:
                    dve.wait_ge(st[od], 16 * u14)  # outb slot reusable
                dve.tensor_tensor(
                    out=mslot(s),
                    in0=oslot(s)[:, :, None].broadcast_to([128, S, 16]),
                    in1=iota_sb[:, None, :].broadcast_to([128, S, 16]),
                    op=mybir.AluOpType.is_equal,
                )

            build_mask(0)
            for b in range(NT):
                s, u = b % DEPTH, b // DEPTH
                if b + 1 < NT:
                    build_mask(b + 1)
                dve.wait_ge(g[s], 16 * (u + 1))
                ob = oslot_out(b % ODEPTH)
                gl = glslot(s)
                mk = mslot(s)
                for o in range(16):
                    ins = dve.copy_predicated(
                        ob[:, :, PROC_DIM:OUT_DIM],
                        mk[:, :, o, None].broadcast_to([128, S, 3]),
                        gl[:, :, 4 * o:4 * o + 3],
                    )
                ins.then_inc(v[s], 1)

    from concourse.library_overlay import lower_extended_insts

    lower_extended_insts(nc)
    return nc


_nc_cache = {}

# test.py reads this for exec_time_ns / trace info after a traced run.
_last_results = None


def _get_nc():
    if "nc" not in _nc_cache:
        _nc_cache["nc"] = build_nc()
    return _nc_cache["nc"]


def _prep_indices(vals, dtype):
    """[NT*T] row-major -> [NT, 128, T//16] wrapped+replicated gather lists.

    Tile row r (= p*S + s) must sit at gather list position j = s*128 + p;
    the wrapped layout stores position j at [j%16, j//16], replicated to all
    8 16-partition groups so every SWDGE queue's q7 pair finds them.
    """
    a = vals.reshape(NT, 128, S)  # [b, p, s]
    a = a.transpose(0, 2, 1).reshape(NT, T // 16, 16)  # [b, j//16, j%16]
    a = a.transpose(0, 2, 1)  # [b, 16, T//16]
    return np.broadcast_to(a[:, None, :, :], (NT, 8, 16, T // 16)).reshape(
        NT, 128, T // 16).astype(dtype)


def kernel(proc_pos, locs_sp, process_ids, location_ids):
    global _last_results
    proc_pos = np.ascontiguousarray(np.asarray(proc_pos, dtype=np.float32))
    locs_sp = np.ascontiguousarray(np.asarray(locs_sp, dtype=np.float32))
    pids = np.asarray(process_ids).astype(np.int32, copy=False)
    lids = np.asarray(location_ids).astype(np.int32, copy=False)

    loc_pad = np.zeros((NBLK * 16, 4), np.float32)
    loc_pad[:NUM_LOCS, :SPATIAL_DIM] = locs_sp
    loc64 = loc_pad.reshape(NBLK, 64)
    # Block-diag rhs: pmat[16g+k, 8g+d] = proc_pos[k, d]
    pmat = np.zeros((128, 64), np.float32)
    for gg in range(8):
        pmat[16 * gg:16 * gg + NUM_PROCS, 8 * gg:8 * gg + PROC_DIM] = proc_pos
    iota16 = np.tile(np.arange(16, dtype=np.int32), (128, 1))

    nc = _get_nc()
    in_maps = []
    for c in range(N_CORES):
        lo, hi = c * PER_CORE, (c + 1) * PER_CORE
        lid_c = np.zeros(N_PAD, np.int32)
        pid_c = np.zeros(N_PAD, np.int32)
        lid_c[:PER_CORE] = lids[lo:hi]
        pid_c[:PER_CORE] = pids[lo:hi]
        # One-hot stationary weights: ohw[b, G, 16g+k, p] = (pid[b,p,8G+g]==k)
        P = pid_c.reshape(NT, 128, 8, 8)  # [b, p, G, g]
        oh = (P[:, :, :, :, None] == np.arange(16, dtype=np.int32)).astype(np.float32)
        # [b, p, G, g, k] -> [b, G, (g,k), p] -> [b, (G,g,k)=8*128, p]... PE wants
        # lhsT partition dim = (g,k), free dim = p: store as [b, 128part=(g,k), 8G*... ]
        # SBUF layout [128, 8*128]: partition = kk=(16g+k), col = G*128 + p.
        ohw_c = np.ascontiguousarray(
            oh.transpose(0, 3, 4, 2, 1)  # [b, g, k, G, p]
            .reshape(NT, 128, 8, 128)    # [b, kk, G, p]
            .reshape(NT, 128, 8 * 128)
        )
        in_maps.append(
            {
                "loc64": loc64,
                "pmat": pmat,
                "iota16": iota16,
                "eidx": _prep_indices(lid_c >> 4, np.int16),
                "ohw": ohw_c,
                "off": (lid_c & 15).astype(np.int32).reshape(NT, 128, S),
            }
        )

    res = run_bass_kernel_spmd(nc, in_maps, list(range(N_CORES)))
    _last_results = res
    out = np.concatenate([r["out"][:PER_CORE] for r in res.results], axis=0)
    return out
